# revision 25
# baseline (speedup 1.0000x reference)
#!/usr/bin/env python3
"""Trainium2 Bass kernel for AdvancedGNNLinkPredictor (3-layer GCN + MLP edge decoder).

Strategy (8 NeuronCores, SPMD):
  - Nodes sharded contiguously across cores (12544 rows each, 128-padded).
  - Per layer: each core computes hW' = dinv * (h @ W_folded) for its shard,
    AllGathers a bf16 node-feature table [100352, 128], then performs edge
    message passing for edges whose dst it owns: dma_gather of source rows +
    segment-sum via one-hot S-matrix matmuls on the TensorEngine (PSUM
    accumulation), then a fused BN/bias/ReLU epilogue.
  - Edge segment-sum avoids dma_scatter_add entirely (its CCE read-modify-write
    races on duplicate indices on HW).
  - Decoder: label edges bucketed by (src-chunk, dst-chunk) so int16 gather
    indices fit; transpose-mode dma_gather produces feature-major z vectors
    fed straight into a 4-layer bf16 matmul MLP.
  - BatchNorm (eval mode) folded into weights/biases on the host.
"""
import sys
import numpy as np

for _p in ("/opt/trn_rl_repo", "/root/.axon_site/_ro/trn_rl_repo"):
    if _p not in sys.path:
        sys.path.append(_p)

from concourse import bass, bacc, tile, mybir  # noqa: E402

BF16_NP = mybir.dt.np(mybir.dt.bfloat16)
F32 = mybir.dt.float32
BF16 = mybir.dt.bfloat16
I16 = mybir.dt.int16


class CFG:
    def __init__(self, N, E, EL, DIN, H, NC=8, CHUNK=32768, TILE=64,
                 GCALL=2048, DSTEP=512, BN_EPS=1e-5):
        self.N, self.E, self.EL, self.DIN, self.H = N, E, EL, DIN, H
        self.NC, self.CHUNK, self.TILE = NC, CHUNK, TILE
        self.GCALL, self.DSTEP, self.BN_EPS = GCALL, DSTEP, BN_EPS
        self.SHARD = -(-N // NC // 128) * 128
        self.NPAD = NC * self.SHARD
        self.NT = self.SHARD // TILE       # 64-node tiles per core
        self.NT2 = self.SHARD // 128       # 128-node tiles per core
        self.NCH = -(-self.NPAD // CHUNK)  # chunks in the gathered table
        self.ELC = EL // NC
        assert EL % NC == 0
        assert self.SHARD % 128 == 0 and TILE == 64


FULL_CFG = dict(N=100_000, E=1_250_000, EL=200_000, DIN=128, H=64)


def _wrap_idx(vals):
    """[n] int -> [128, n/16] int16 SBUF gather-index layout (i -> [i%16,i//16],
    replicated 8x across partition groups for the Q7 cpus)."""
    n = len(vals)
    assert n % 16 == 0
    a = np.asarray(vals, np.int16).reshape(n // 16, 16).T.copy()
    return np.tile(a, (8, 1))


def prepare(cfg, inp):
    """Host-side preprocessing. Returns (sched, percore) where sched is the
    core-independent static schedule and percore is a list of per-core input
    dicts for the device kernel."""
    f32 = np.float32
    N, E, NC = cfg.N, cfg.E, cfg.NC
    SHARD, CHUNK, TILE, NT, NCH = cfg.SHARD, cfg.CHUNK, cfg.TILE, cfg.NT, cfg.NCH

    x = np.asarray(inp["x"], f32)
    ei = np.asarray(inp["edge_index"], np.int64)
    eli = np.asarray(inp["edge_label_index"], np.int64)

    # ---- fold BN into GCN weights
    Ws, ub = [], []
    for l in range(3):
        W = np.asarray(inp[f"W{l}"], f32)
        b = np.asarray(inp[f"b{l}"], f32)
        g = np.asarray(inp[f"g{l}"], f32)
        be = np.asarray(inp[f"be{l}"], f32)
        m = np.asarray(inp[f"m{l}"], f32)
        v = np.asarray(inp[f"v{l}"], f32)
        s = g / np.sqrt(v + cfg.BN_EPS)
        Ws.append((W * s[None, :]).astype(f32))
        ub.append(((b - m) * s + be).astype(f32))

    # ---- decoder folds
    pW1 = np.asarray(inp["pW1"], f32); pb1 = np.asarray(inp["pb1"], f32)
    pW2 = np.asarray(inp["pW2"], f32); pb2 = np.asarray(inp["pb2"], f32)
    pW3 = np.asarray(inp["pW3"], f32); pb3 = np.asarray(inp["pb3"], f32)
    pW4 = np.asarray(inp["pW4"], f32); pb4 = np.asarray(inp["pb4"], f32)
    s1 = np.asarray(inp["pg1"], f32) / np.sqrt(np.asarray(inp["pv1"], f32) + cfg.BN_EPS)
    s2 = np.asarray(inp["pg2"], f32) / np.sqrt(np.asarray(inp["pv2"], f32) + cfg.BN_EPS)
    dW1 = pW1 * s1[None, :]
    du1 = (pb1 - np.asarray(inp["pm1"], f32)) * s1 + np.asarray(inp["pbe1"], f32)
    dW2 = pW2 * s2[None, :]
    du2 = (pb2 - np.asarray(inp["pm2"], f32)) * s2 + np.asarray(inp["pbe2"], f32)
    dW3, du3 = pW3, pb3
    dW4, db4 = pW4, float(pb4.reshape(-1)[0])
    H2 = 2 * cfg.H
    dW1i = np.zeros((H2, H2), f32); dW1i[:cfg.H] = dW1[:cfg.H]
    dW1j = np.zeros((H2, H2), f32); dW1j[:cfg.H] = dW1[cfg.H:]

    # ---- degrees
    src, dst = ei[0], ei[1]
    deg = 1.0 + np.bincount(dst, minlength=N).astype(f32)
    dinv = (1.0 / np.sqrt(deg)).astype(f32)
    dinv_pad = np.zeros(cfg.NPAD, f32)
    dinv_pad[:N] = dinv

    # ---- table-row remap: row = (m >= SHARD/2)*NPAD/2 + owner*SHARD/2 + m%(SHARD/2)
    # so the AllGather can be split into two half-shard collectives whose
    # outputs are contiguous table halves (half A = rows [0, NPAD/2)).
    S2 = SHARD // 2

    def remap(n):
        r, m = n // SHARD, n % SHARD
        return (m >= S2) * (cfg.NPAD // 2) + r * S2 + (m % S2)

    # ---- per-core edge lists sorted by (chunk, tile, src)
    # Chunks 0..NCH-2 aggregate into 64-node tiles; the last chunk (few source
    # rows -> tiny segments) uses 128-node tiles to halve pair count.
    wid_of = [TILE] * NCH
    if NCH >= 2:
        wid_of[NCH - 1] = 128
    ntile_of = [SHARD // w for w in wid_of]
    owner = dst // SHARD
    ek, et, esl, ecol = [], [], [], []
    counts = [np.zeros((NC, ntile_of[k]), np.int64) for k in range(NCH)]
    for c in range(NC):
        sel = owner == c
        s = remap(src[sel])
        dl = dst[sel] - c * SHARD
        k = s // CHUNK
        w = np.array(wid_of, np.int64)[k]
        t = dl // w
        col = dl % w
        order = np.lexsort((s, t, k))
        ek.append(k[order]); et.append(t[order])
        esl.append((s % CHUNK)[order].astype(np.int16))
        ecol.append(col[order].astype(np.int16))
        for kk in range(NCH):
            m = k[order] == kk
            np.add.at(counts[kk][c], t[order][m], 1)

    L = [counts[k].max(axis=0) for k in range(NCH)]  # ragged [NCH][ntile]
    for k in range(NCH):    # chunk runs must be 128-aligned
        L[k][ntile_of[k] - 1] += (-L[k].sum()) % 128
    # Run order [0, NCH-1, 1, .., NCH-2]: the last run is a large 64-wide one,
    # so first-half tiles finalize well before layer end and the next layer's
    # first half-AllGather hides under the B2 tail.
    run_order = ([0, NCH - 1] + list(range(1, NCH - 1))) if NCH >= 2 else [0]
    seg_off = [np.zeros(ntile_of[k], np.int64) for k in range(NCH)]
    run_rng = {}
    pos = 0
    for k in run_order:
        a = pos
        for t in range(ntile_of[k]):
            seg_off[k][t] = pos
            pos += L[k][t]
        run_rng[k] = (a, pos)
    E_pad = int(pos)
    assert E_pad % 128 == 0
    G = E_pad // 128

    # For 64-wide chunks: first non-empty chunk per 64-tile decides copy/add.
    # Tiles never touched by a 64-wide chunk are memset up front; the 128-wide
    # last chunk always accumulates with "add".
    n64 = NCH - 1 if NCH >= 2 else NCH
    first_k = np.full(NT, -1, np.int64)
    for t in range(NT):
        if L[0][t] > 0:
            first_k[t] = 0

    # ---- pairs (g, k, t, start, stop, wid), sorted by stream position
    pairs = []
    for k in range(NCH):
        for t in range(ntile_of[k]):
            if L[k][t] == 0:
                continue
            p0 = seg_off[k][t]; p1 = p0 + L[k][t]
            g0, g1 = p0 // 128, (p1 - 1) // 128
            for g in range(g0, g1 + 1):
                pairs.append((int(g), k, t, g == g0, g == g1, wid_of[k]))
    pairs.sort(key=lambda p: (p[0], seg_off[p[1]][p[2]]))
    NPAIRS = len(pairs)
    pair_g = np.array([p[0] for p in pairs], np.int64)
    pair_w = np.array([p[5] for p in pairs], np.int64)
    pair_off = np.concatenate([[0], np.cumsum(pair_w)]).astype(np.int64)
    SCOLS = int(pair_off[-1])

    # map every stream position to its pair id
    pair_of_pos = np.full(E_pad, -1, np.int64)
    for pi, (g, k, t, st, sp, w) in enumerate(pairs):
        a = max(g * 128, seg_off[k][t])
        b = min((g + 1) * 128, seg_off[k][t] + L[k][t])
        pair_of_pos[a:b] = pi

    # ---- per-core streams + S one-hot data
    percore_stream = []
    e_local = np.arange(E_pad) % 128
    for c in range(NC):
        idx_stream = np.zeros(E_pad, np.int16)
        scol = np.full(E_pad, -1, np.int16)
        ptr = np.zeros((NCH, NT), np.int64)
        # place this core's (already sorted) edges segment by segment
        kk, tt = ek[c], et[c]
        seg_id = kk * NT + tt
        # edges are sorted by (k,t) so contiguous runs per segment
        boundaries = np.nonzero(np.diff(seg_id))[0] + 1
        starts = np.concatenate([[0], boundaries])
        ends = np.concatenate([boundaries, [len(seg_id)]])
        for a, b in zip(starts, ends):
            if a == b:
                continue
            k, t = int(kk[a]), int(tt[a])
            o = seg_off[k][t]
            idx_stream[o:o + (b - a)] = esl[c][a:b]
            scol[o:o + (b - a)] = ecol[c][a:b]
        S = np.zeros((128, SCOLS), BF16_NP)
        valid = scol >= 0
        S[e_local[valid], pair_off[pair_of_pos[valid]] + scol[valid]] = 1
        percore_stream.append((idx_stream, S))

    # ---- gather calls
    calls = []  # (k, pos0, n, pair_lo, pair_hi)
    for k in run_order:
        a, b = run_rng[k]
        for off in range(a, b, cfg.GCALL):
            n = min(cfg.GCALL, b - off)
            g0, g1 = off // 128, (off + n) // 128
            plo = int(np.searchsorted(pair_g, g0, "left"))
            phi = int(np.searchsorted(pair_g, g1, "left"))
            calls.append((k, off, n, plo, phi))

    # ---- decoder buckets
    li, lj = eli[0], eli[1]
    bi, bj, bperm = [], [], []
    bcounts = np.zeros((NC, NCH * NCH), np.int64)
    for c in range(NC):
        ii = remap(li[c * cfg.ELC:(c + 1) * cfg.ELC])
        jj = remap(lj[c * cfg.ELC:(c + 1) * cfg.ELC])
        bb = (ii // CHUNK) * NCH + (jj // CHUNK)
        order = np.argsort(bb, kind="stable")
        bi.append((ii % CHUNK)[order].astype(np.int16))
        bj.append((jj % CHUNK)[order].astype(np.int16))
        bperm.append(order + c * cfg.ELC)
        np.add.at(bcounts[c], bb[order], 1)
    LB = bcounts.max(axis=0)
    LB = ((LB + 127) // 128) * 128 * (LB > 0)
    ELPAD = int(LB.sum())
    bucket_off = np.concatenate([[0], np.cumsum(LB)]).astype(np.int64)

    deci, decj, perm = [], [], []
    for c in range(NC):
        di = np.zeros(ELPAD, np.int16)
        dj = np.zeros(ELPAD, np.int16)
        pm = np.full(ELPAD, -1, np.int64)
        csum = np.concatenate([[0], np.cumsum(bcounts[c])]).astype(np.int64)
        for b in range(NCH * NCH):
            nb = int(bcounts[c][b])
            if nb == 0:
                continue
            o = int(bucket_off[b])
            di[o:o + nb] = bi[c][csum[b]:csum[b] + nb]
            dj[o:o + nb] = bj[c][csum[b]:csum[b] + nb]
            pm[o:o + nb] = bperm[c][csum[b]:csum[b] + nb]
        deci.append(di); decj.append(dj); perm.append(pm)

    dbuckets = []  # (ci, cj, off, Lb)
    for b in range(NCH * NCH):
        if LB[b]:
            dbuckets.append((b // NCH, b % NCH, int(bucket_off[b]), int(LB[b])))

    # per-128-node-tile: the pair whose flush finalizes its accumulator
    NT2 = cfg.NT2
    stop_pair_of_seg = {}
    for pi, (g, k, t, st, sp, w) in enumerate(pairs):
        if sp:
            stop_pair_of_seg[(k, t)] = pi
    b3_after = {}   # pair index -> list of nt tiles finalized by that flush
    b3_empty = []   # tiles with no edges anywhere (memset path only)
    for nt in range(NT2):
        best = -1
        for k in range(NCH):
            if wid_of[k] == 128:
                if L[k][nt] > 0:
                    best = max(best, stop_pair_of_seg[(k, nt)])
            else:
                for t in (2 * nt, 2 * nt + 1):
                    if L[k][t] > 0:
                        best = max(best, stop_pair_of_seg[(k, t)])
        if best < 0:
            b3_empty.append(nt)
        else:
            b3_after.setdefault(best, []).append(nt)

    sched = dict(E_pad=E_pad, G=G, NPAIRS=NPAIRS, pairs=pairs, calls=calls,
                 first_k=first_k, L=L, seg_off=seg_off, pair_off=pair_off,
                 SCOLS=SCOLS, ELPAD=ELPAD, dbuckets=dbuckets, db4=db4,
                 b3_after=b3_after, b3_empty=b3_empty)

    # ---- per-core device input dicts
    xpad = np.zeros((cfg.NPAD, cfg.DIN), f32)
    xpad[:N] = x
    percore = []
    for c in range(NC):
        d = {}
        cs = c * SHARD
        d["xT"] = np.ascontiguousarray(xpad[cs:cs + SHARD].T)          # [DIN, SHARD]
        d["dinv128"] = np.ascontiguousarray(
            dinv_pad[cs:cs + SHARD].reshape(cfg.NT2, 128).T)           # [128, NT2]
        d["gidx"] = _wrap_idx(percore_stream[c][0])                    # [128, E_pad/16]
        d["S"] = percore_stream[c][1]                                  # [128, NPAIRS*64]
        for l in range(3):
            d[f"Ws{l}"] = Ws[l]
            d[f"u{l}"] = np.tile(ub[l][None, :], (128, 1)).astype(f32)  # [128, H]
        d["dW1i"] = dW1i.astype(BF16_NP)
        d["dW1j"] = dW1j.astype(BF16_NP)
        d["dW2"] = dW2.astype(BF16_NP)
        d["dW3"] = dW3.astype(BF16_NP)
        d["dW4"] = dW4.astype(BF16_NP)
        d["du1"] = du1.reshape(-1, 1).astype(f32)
        d["du2"] = du2.reshape(-1, 1).astype(f32)
        d["du3"] = du3.reshape(-1, 1).astype(f32)
        d["deci"] = _wrap_idx(deci[c])
        d["decj"] = _wrap_idx(decj[c])
        d["ident"] = np.eye(128, dtype=f32)
        percore.append(d)

    return sched, percore, perm


def build(cfg, sched):
    NC, H, DIN, SHARD, NT2, CHUNK, NCH = (cfg.NC, cfg.H, cfg.DIN, cfg.SHARD,
                                          cfg.NT2, cfg.CHUNK, cfg.NCH)
    E_pad, NPAIRS, ELPAD = sched["E_pad"], sched["NPAIRS"], sched["ELPAD"]
    TILE = cfg.TILE
    H2 = 2 * H

    nc = bacc.Bacc(None, target_bir_lowering=False, debug=False,
                   num_swdge_queues=4)

    # ---- dram parameters
    P = {}
    def di(name, shape, dtype):
        P[name] = nc.dram_tensor(name, list(shape), dtype, kind="ExternalInput")
    di("xT", (DIN, SHARD), F32)
    di("dinv128", (128, NT2), F32)
    di("gidx", (128, E_pad // 16), I16)
    di("S", (128, sched["SCOLS"]), BF16)
    di("Ws0", (DIN, H), F32); di("Ws1", (H, H), F32); di("Ws2", (H, H), F32)
    di("u0", (128, H), F32); di("u1", (128, H), F32); di("u2", (128, H), F32)
    di("dW1i", (H2, H2), BF16); di("dW1j", (H2, H2), BF16)
    di("dW2", (H2, H), BF16); di("dW3", (H, H // 2), BF16); di("dW4", (H // 2, 1), BF16)
    di("du1", (H2, 1), F32); di("du2", (H, 1), F32); di("du3", (H // 2, 1), F32)
    di("deci", (128, ELPAD // 16), I16)
    di("decj", (128, ELPAD // 16), I16)
    di("ident", (128, 128), F32)
    out = nc.dram_tensor("out", [1, ELPAD], F32, kind="ExternalOutput")

    bounce = nc.dram_tensor("bounce", [SHARD, 128], BF16)
    table = nc.dram_tensor("table", [cfg.NPAD, 128], BF16, addr_space="Shared")
    bounce_re = bounce.ap().rearrange("(t p) f -> p t f", p=128)  # [128, NT2, 128]

    def chunk_rows(k):
        lo = k * CHUNK
        hi = min((k + 1) * CHUNK, cfg.NPAD)
        return table[lo:hi, :]

    LBMAX = max((b[3] for b in sched["dbuckets"]), default=128)

    with tile.TileContext(nc) as tc:
        with tc.tile_pool(name="res", bufs=1) as res, \
             tc.tile_pool(name="sb", bufs=2) as sbp, \
             tc.tile_pool(name="sb3", bufs=4) as sbp3, \
             tc.tile_pool(name="pm", bufs=2, space="PSUM") as pmain, \
             tc.tile_pool(name="pae", bufs=3, space="PSUM") as pacc_e, \
             tc.tile_pool(name="pao", bufs=3, space="PSUM") as pacc_o:

            gq = [0]
            # ---- residents
            h_node = res.tile([128, NT2, H], F32)
            acc = res.tile([128, NT2, H], F32)
            hWloc = res.tile([128, NT2, H], F32)
            hW_bf = res.tile([128, NT2, 128], BF16)
            dinv_sb = res.tile([128, NT2], F32)
            gidx_sb = res.tile([128, E_pad // 16], I16)
            ident_sb = res.tile([128, 128], F32)
            Ws_sb = [res.tile([DIN if l == 0 else H, H], F32, name=f"Ws{l}_sb") for l in range(3)]
            u_sb = [res.tile([128, H], F32, name=f"u{l}_sb") for l in range(3)]
            dW1i_sb = res.tile([H2, H2], BF16)
            dW1j_sb = res.tile([H2, H2], BF16)
            dW2_sb = res.tile([H2, H], BF16)
            dW3_sb = res.tile([H, H // 2], BF16)
            dW4_sb = res.tile([H // 2, 1], BF16)
            du1_sb = res.tile([H2, 1], F32)
            du2_sb = res.tile([H, 1], F32)
            du3_sb = res.tile([H // 2, 1], F32)

            nc.sync.dma_start(out=dinv_sb[:], in_=P["dinv128"][:])
            nc.sync.dma_start(out=gidx_sb[:], in_=P["gidx"][:])
            nc.sync.dma_start(out=ident_sb[:], in_=P["ident"][:])
            for l in range(3):
                nc.sync.dma_start(out=Ws_sb[l][:], in_=P[f"Ws{l}"][:])
                nc.sync.dma_start(out=u_sb[l][:], in_=P[f"u{l}"][:])
            for t_, n_ in ((dW1i_sb, "dW1i"), (dW1j_sb, "dW1j"), (dW2_sb, "dW2"),
                           (dW3_sb, "dW3"), (dW4_sb, "dW4"), (du1_sb, "du1"),
                           (du2_sb, "du2"), (du3_sb, "du3")):
                nc.sync.dma_start(out=t_[:], in_=P[n_][:])
            nc.vector.memset(hW_bf[:], 0.0)

            ACT = mybir.ActivationFunctionType

            def finish_b0(nt, psB):
                nc.vector.tensor_scalar_mul(hWloc[:, nt, :], psB[:],
                                            dinv_sb[:, nt:nt + 1])
                nc.scalar.activation(hW_bf[:, nt, 0:H], hWloc[:, nt, :], ACT.Copy)

            first_k = sched["first_k"]

            def b0_tile(l_, nt):
                """hW'(l_) for 128-node tile nt (transpose path, l_ >= 1)."""
                psT = pmain.tile([H, 128], F32, tag="gen", name=f"psT{l_}_{nt}")
                nc.tensor.transpose(psT[:], h_node[:, nt, :], ident_sb[:])
                hTt = sbp.tile([H, 128], F32, tag="hTt", name=f"hTt{l_}_{nt}")
                nc.scalar.activation(hTt[:], psT[:], ACT.Copy)
                psB = pmain.tile([128, H], F32, tag="gen", name=f"psB{l_}_{nt}")
                nc.tensor.matmul(psB[:], hTt[:], Ws_sb[l_][:],
                                 start=True, stop=True)
                finish_b0(nt, psB)

            def b3_tile(l_, nt):
                """Epilogue for 128-node tile nt of layer l_."""
                tmp = sbp.tile([128, H], F32, tag="ep", name=f"ep{l_}_{nt}")
                nc.vector.tensor_add(tmp[:], acc[:, nt, :], hWloc[:, nt, :])
                nc.vector.tensor_scalar_mul(tmp[:], tmp[:], dinv_sb[:, nt:nt + 1])
                nc.vector.tensor_add(tmp[:], tmp[:], u_sb[l_][:])
                if l_ < 2:
                    nc.vector.tensor_scalar_max(h_node[:, nt, :], tmp[:], 0.0)
                else:
                    nc.scalar.activation(hW_bf[:, nt, 0:H], tmp[:], ACT.Copy)

            fin_cnt = [0]

            def finalize_tile(l_, nt):
                b3_tile(l_, nt)
                if l_ < 2:
                    b0_tile(l_ + 1, nt)
                if nt < NT2 // 2:
                    fin_cnt[0] += 1
                    if fin_cnt[0] == NT2 // 2:
                        ag_half(0)  # next table's first half is ready

            # ---- B0 layer 0: streamed xT slabs (emitted after ag_half is
            # defined below via a deferred list)
            _layer0_b0 = []

            def emit_layer0_b0():
                SLAB = 5  # slabs of SLAB 128-tiles
                for s0 in range(0, NT2, SLAB):
                    sw = min(SLAB, NT2 - s0)
                    xsl = sbp.tile([DIN, 5 * 128], F32, tag="big",
                                   name=f"xsl{s0}")
                    nc.sync.dma_start(out=xsl[:, 0:sw * 128],
                                      in_=P["xT"][:, s0 * 128:(s0 + sw) * 128])
                    for j in range(sw):
                        nt = s0 + j
                        psB = pmain.tile([128, H], F32, tag="gen",
                                         name=f"psB0_{nt}")
                        nc.tensor.matmul(psB[:], xsl[:, j * 128:(j + 1) * 128],
                                         Ws_sb[0][:], start=True, stop=True)
                        finish_b0(nt, psB)
                        if nt == NT2 // 2 - 1:
                            ag_half(0)
                ag_half(1)

            b3_after, b3_empty = sched["b3_after"], sched["b3_empty"]
            NTH = NT2 // 2          # tiles in the first table half
            S2R = NTH * 128         # bounce rows per half
            NP2 = cfg.NPAD // 2

            def ag_half(h):
                lo = h * NTH
                hi = NT2 if h else NTH
                nc.sync.dma_start(out=bounce_re[:, lo:hi, :],
                                  in_=hW_bf[:, lo:hi, :])
                nc.gpsimd.collective_compute(
                    "AllGather", mybir.AluOpType.bypass,
                    replica_groups=[list(range(NC))],
                    ins=[bounce[h * S2R:h * S2R + (hi - lo) * 128, :].opt()],
                    outs=[table[h * NP2:h * NP2 + NC * (hi - lo) * 128, :].opt()])

            emit_layer0_b0()

            for l in range(3):
                # (the table AGs for this layer were issued during the previous
                # layer's B2 / the layer-0 slab loop)
                # zero accumulator halves never touched by a 64-wide chunk
                for t in range(cfg.NT):
                    if first_k[t] < 0:
                        half, nt = t & 1, t >> 1
                        nc.vector.memset(acc[half * 64:half * 64 + 64, nt, :], 0.0)
                fin_cnt[0] = 0
                for nt in b3_empty:  # tiles with no edges at all
                    finalize_tile(l, nt)

                # ---- B2: message passing (B3 + next-layer B0 fire per tile as
                # soon as its accumulator is final, overlapping the B2 tail)
                cur = {}
                for ci_, (k, pos0, n, plo, phi) in enumerate(sched["calls"]):
                    msg = sbp3.tile([128, cfg.GCALL // 128, 128], BF16, tag="msg")
                    nc.gpsimd.dma_gather(
                        out_ap=msg[:, 0:n // 128, :], in_ap=chunk_rows(k),
                        idxs_ap=gidx_sb[:, pos0 // 16:(pos0 + n) // 16],
                        num_idxs=n, num_idxs_reg=n,
                        elem_size=128, single_packet=False,
                        queue_num=gq[0] % 4); gq[0] += 1
                    SWCOLS = 56 * TILE  # S-window column budget
                    po = sched["pair_off"]
                    win_lo = plo
                    Ssb = None
                    col_lo = 0
                    for li_ in range(phi - plo):
                        pi = plo + li_
                        g, k2, t, st, sp, wid = sched["pairs"][pi]
                        if Ssb is None or int(po[pi + 1] - col_lo) > SWCOLS:
                            col_lo = int(po[pi])
                            # window covers pairs [pi, wend)
                            wend = pi
                            while (wend < phi
                                   and int(po[wend + 1] - col_lo) <= SWCOLS):
                                wend += 1
                            ncols = int(po[wend] - col_lo)
                            Ssb = sbp.tile([128, SWCOLS], BF16, tag="Swin",
                                           name=f"Ssb{pi}")
                            nc.sync.dma_start(out=Ssb[:, 0:ncols],
                                              in_=P["S"][:, col_lo:col_lo + ncols])
                        gl = g - pos0 // 128
                        so = int(po[pi] - col_lo)
                        if wid == 64:
                            half, nt = t & 1, t >> 1
                            if ("h", t) not in cur:
                                pool_ = pacc_e if half == 0 else pacc_o
                                cur[("h", t)] = pool_.tile(
                                    [128, H], F32, name=f"pacc{t}",
                                    tag="ae" if half == 0 else "ao")
                            ps = cur[("h", t)]
                            pss = ps[half * 64:half * 64 + 64, :]
                            nc.tensor.matmul(pss, Ssb[:, so:so + 64],
                                             msg[:, gl, 0:H], start=st, stop=sp)
                            if sp:
                                asl = acc[half * 64:half * 64 + 64, nt, :]
                                if first_k[t] == k:
                                    nc.scalar.activation(asl, pss, ACT.Copy)
                                else:
                                    nc.vector.tensor_add(asl, asl, pss)
                                del cur[("h", t)]
                                for nt_ in b3_after.get(pi, []):
                                    finalize_tile(l, nt_)
                        else:
                            if ("f", t) not in cur:
                                cur[("f", t)] = pacc_e.tile(
                                    [128, H], F32, name=f"paccf{t}", tag="ae")
                            ps = cur[("f", t)]
                            nc.tensor.matmul(ps[:], Ssb[:, so:so + 128],
                                             msg[:, gl, 0:H], start=st, stop=sp)
                            if sp:
                                nc.vector.tensor_add(acc[:, t, :], acc[:, t, :],
                                                     ps[:])
                                del cur[("f", t)]
                                for nt_ in b3_after.get(pi, []):
                                    finalize_tile(l, nt_)
                assert not cur
                ag_half(1)  # next table's second half

            # ---- decoder
            for bi_, (ci, cj, off, Lb) in enumerate(sched["dbuckets"]):
                zti = sbp.tile([128, 1, LBMAX], BF16, tag="zti", name=f"zti{bi_}")
                ztj = sbp.tile([128, 1, LBMAX], BF16, tag="ztj", name=f"ztj{bi_}")
                DGC = 2048
                for zt, idx_dram, ck in ((zti, P["deci"], ci), (ztj, P["decj"], cj)):
                    for s0 in range(0, Lb, DGC):
                        n0 = min(DGC, Lb - s0)
                        isl = sbp3.tile([128, DGC // 16], I16, tag="gidxw",
                                        name=f"isl{off}_{s0}")
                        nc.sync.dma_start(
                            out=isl[:, 0:n0 // 16],
                            in_=idx_dram[:, (off + s0) // 16:(off + s0 + n0) // 16])
                        nc.gpsimd.dma_gather(
                            out_ap=zt[:, :, s0:s0 + n0], in_ap=chunk_rows(ck),
                            idxs_ap=isl[:, 0:n0 // 16], num_idxs=n0, num_idxs_reg=n0,
                            elem_size=128, transpose=True, single_packet=False,
                            queue_num=gq[0] % 4); gq[0] += 1
                for s in range(0, Lb, cfg.DSTEP):
                    w = min(cfg.DSTEP, Lb - s)
                    ps1 = pmain.tile([128, cfg.DSTEP], F32, tag="gen")
                    nc.tensor.matmul(ps1[:, 0:w], dW1i_sb[:], zti[:, 0, s:s + w],
                                     start=True, stop=False)
                    nc.tensor.matmul(ps1[:, 0:w], dW1j_sb[:], ztj[:, 0, s:s + w],
                                     start=False, stop=True)
                    a1 = sbp.tile([128, cfg.DSTEP], BF16, tag="a1")
                    nc.scalar.activation(a1[:, 0:w], ps1[:, 0:w], ACT.Relu,
                                         bias=du1_sb[:], scale=1.0)
                    ps2 = pmain.tile([H, cfg.DSTEP], F32, tag="gen")
                    nc.tensor.matmul(ps2[:, 0:w], dW2_sb[:], a1[:, 0:w],
                                     start=True, stop=True)
                    a2 = sbp.tile([H, cfg.DSTEP], BF16, tag="a2")
                    nc.scalar.activation(a2[:, 0:w], ps2[:, 0:w], ACT.Relu,
                                         bias=du2_sb[:], scale=1.0)
                    ps3 = pmain.tile([H // 2, cfg.DSTEP], F32, tag="gen")
                    nc.tensor.matmul(ps3[:, 0:w], dW3_sb[:], a2[:, 0:w],
                                     start=True, stop=True)
                    a3 = sbp.tile([H // 2, cfg.DSTEP], BF16, tag="a3")
                    nc.scalar.activation(a3[:, 0:w], ps3[:, 0:w], ACT.Relu,
                                         bias=du3_sb[:], scale=1.0)
                    ps4 = pmain.tile([1, cfg.DSTEP], F32, tag="gen")
                    nc.tensor.matmul(ps4[:, 0:w], dW4_sb[:], a3[:, 0:w],
                                     start=True, stop=True)
                    o_ = sbp.tile([1, cfg.DSTEP], F32, tag="od")
                    nc.scalar.activation(o_[:, 0:w], ps4[:, 0:w], ACT.Copy,
                                         bias=float(sched["db4"]))
                    nc.sync.dma_start(out=out[0:1, off + s:off + s + w],
                                      in_=o_[:, 0:w])

    nc.compile()
    return nc


def unshard(cfg, sched, perm, results):
    res = np.zeros(cfg.EL, np.float32)
    for c in range(cfg.NC):
        o = np.asarray(results[c]["out"], np.float32).reshape(-1)
        mask = perm[c] >= 0
        res[perm[c][mask]] = o[mask]
    return res


def kernel(**inputs):
    from concourse import bass_utils
    cfg = CFG(**FULL_CFG)
    sched, percore, perm = prepare(cfg, inputs)
    nc = build(cfg, sched)
    res = bass_utils.run_bass_kernel_spmd(
        nc, percore, core_ids=list(range(cfg.NC)))
    return unshard(cfg, sched, perm, res.results)


if __name__ == "__main__":
    print("kernel module; use test.py")


# revision 27
# speedup vs baseline: 1.0890x; 1.0890x over previous
#!/usr/bin/env python3
"""Trainium2 Bass kernel for AdvancedGNNLinkPredictor (3-layer GCN + MLP edge decoder).

Strategy (8 NeuronCores, SPMD):
  - Nodes sharded contiguously across cores (12544 rows each, 128-padded).
  - Per layer: each core computes hW' = dinv * (h @ W_folded) for its shard,
    AllGathers a bf16 node-feature table [100352, 128], then performs edge
    message passing for edges whose dst it owns: dma_gather of source rows +
    segment-sum via one-hot S-matrix matmuls on the TensorEngine (PSUM
    accumulation), then a fused BN/bias/ReLU epilogue.
  - Edge segment-sum avoids dma_scatter_add entirely (its CCE read-modify-write
    races on duplicate indices on HW).
  - Decoder: label edges bucketed by (src-chunk, dst-chunk) so int16 gather
    indices fit; transpose-mode dma_gather produces feature-major z vectors
    fed straight into a 4-layer bf16 matmul MLP.
  - BatchNorm (eval mode) folded into weights/biases on the host.
"""
import sys
import numpy as np

for _p in ("/opt/trn_rl_repo", "/root/.axon_site/_ro/trn_rl_repo"):
    if _p not in sys.path:
        sys.path.append(_p)

from concourse import bass, bacc, tile, mybir  # noqa: E402

BF16_NP = mybir.dt.np(mybir.dt.bfloat16)
F32 = mybir.dt.float32
BF16 = mybir.dt.bfloat16
I16 = mybir.dt.int16


class CFG:
    def __init__(self, N, E, EL, DIN, H, NC=8, CHUNK=32768, TILE=64,
                 GCALL=2048, DSTEP=512, BN_EPS=1e-5):
        self.N, self.E, self.EL, self.DIN, self.H = N, E, EL, DIN, H
        self.NC, self.CHUNK, self.TILE = NC, CHUNK, TILE
        self.GCALL, self.DSTEP, self.BN_EPS = GCALL, DSTEP, BN_EPS
        self.SHARD = -(-N // NC // 128) * 128
        self.NPAD = NC * self.SHARD
        self.NT = self.SHARD // TILE       # 64-node tiles per core
        self.NT2 = self.SHARD // 128       # 128-node tiles per core
        self.NCH = -(-self.NPAD // CHUNK)  # chunks in the gathered table
        self.ELC = EL // NC
        assert EL % NC == 0
        assert self.SHARD % 128 == 0 and TILE == 64


FULL_CFG = dict(N=100_000, E=1_250_000, EL=200_000, DIN=128, H=64)


def _wrap_idx(vals):
    """[n] int -> [128, n/16] int16 SBUF gather-index layout (i -> [i%16,i//16],
    replicated 8x across partition groups for the Q7 cpus)."""
    n = len(vals)
    assert n % 16 == 0
    a = np.asarray(vals, np.int16).reshape(n // 16, 16).T.copy()
    return np.tile(a, (8, 1))


def prepare(cfg, inp):
    """Host-side preprocessing. Returns (sched, percore) where sched is the
    core-independent static schedule and percore is a list of per-core input
    dicts for the device kernel."""
    f32 = np.float32
    N, E, NC = cfg.N, cfg.E, cfg.NC
    SHARD, CHUNK, TILE, NT, NCH = cfg.SHARD, cfg.CHUNK, cfg.TILE, cfg.NT, cfg.NCH

    x = np.asarray(inp["x"], f32)
    ei = np.asarray(inp["edge_index"], np.int64)
    eli = np.asarray(inp["edge_label_index"], np.int64)

    # ---- fold BN into GCN weights
    Ws, ub = [], []
    for l in range(3):
        W = np.asarray(inp[f"W{l}"], f32)
        b = np.asarray(inp[f"b{l}"], f32)
        g = np.asarray(inp[f"g{l}"], f32)
        be = np.asarray(inp[f"be{l}"], f32)
        m = np.asarray(inp[f"m{l}"], f32)
        v = np.asarray(inp[f"v{l}"], f32)
        s = g / np.sqrt(v + cfg.BN_EPS)
        Ws.append((W * s[None, :]).astype(f32))
        ub.append(((b - m) * s + be).astype(f32))

    # ---- decoder folds
    pW1 = np.asarray(inp["pW1"], f32); pb1 = np.asarray(inp["pb1"], f32)
    pW2 = np.asarray(inp["pW2"], f32); pb2 = np.asarray(inp["pb2"], f32)
    pW3 = np.asarray(inp["pW3"], f32); pb3 = np.asarray(inp["pb3"], f32)
    pW4 = np.asarray(inp["pW4"], f32); pb4 = np.asarray(inp["pb4"], f32)
    s1 = np.asarray(inp["pg1"], f32) / np.sqrt(np.asarray(inp["pv1"], f32) + cfg.BN_EPS)
    s2 = np.asarray(inp["pg2"], f32) / np.sqrt(np.asarray(inp["pv2"], f32) + cfg.BN_EPS)
    dW1 = pW1 * s1[None, :]
    du1 = (pb1 - np.asarray(inp["pm1"], f32)) * s1 + np.asarray(inp["pbe1"], f32)
    dW2 = pW2 * s2[None, :]
    du2 = (pb2 - np.asarray(inp["pm2"], f32)) * s2 + np.asarray(inp["pbe2"], f32)
    dW3, du3 = pW3, pb3
    dW4, db4 = pW4, float(pb4.reshape(-1)[0])
    H2 = 2 * cfg.H
    dW1i = np.zeros((H2, H2), f32); dW1i[:cfg.H] = dW1[:cfg.H]
    dW1j = np.zeros((H2, H2), f32); dW1j[:cfg.H] = dW1[cfg.H:]

    # ---- degrees
    src, dst = ei[0], ei[1]
    deg = 1.0 + np.bincount(dst, minlength=N).astype(f32)
    dinv = (1.0 / np.sqrt(deg)).astype(f32)
    dinv_pad = np.zeros(cfg.NPAD, f32)
    dinv_pad[:N] = dinv

    # ---- table-row remap: row = (m >= SHARD/2)*NPAD/2 + owner*SHARD/2 + m%(SHARD/2)
    # so the AllGather can be split into two half-shard collectives whose
    # outputs are contiguous table halves (half A = rows [0, NPAD/2)).
    S2 = SHARD // 2

    def remap(n):
        r, m = n // SHARD, n % SHARD
        return (m >= S2) * (cfg.NPAD // 2) + r * S2 + (m % S2)

    # ---- per-core edge lists sorted by (chunk, tile, src)
    # Chunks 0..NCH-2 aggregate into 64-node tiles; the last chunk (few source
    # rows -> tiny segments) uses 128-node tiles to halve pair count.
    wid_of = [TILE] * NCH
    if NCH >= 2:
        wid_of[NCH - 1] = 128
    ntile_of = [SHARD // w for w in wid_of]
    owner = dst // SHARD
    ek, et, esl, ecol = [], [], [], []
    counts = [np.zeros((NC, ntile_of[k]), np.int64) for k in range(NCH)]
    for c in range(NC):
        sel = owner == c
        s = remap(src[sel])
        dl = dst[sel] - c * SHARD
        k = s // CHUNK
        w = np.array(wid_of, np.int64)[k]
        t = dl // w
        col = dl % w
        order = np.lexsort((s, t, k))
        ek.append(k[order]); et.append(t[order])
        esl.append((s % CHUNK)[order].astype(np.int16))
        ecol.append(col[order].astype(np.int16))
        for kk in range(NCH):
            m = k[order] == kk
            np.add.at(counts[kk][c], t[order][m], 1)

    L = [counts[k].max(axis=0) for k in range(NCH)]  # ragged [NCH][ntile]
    for k in range(NCH):    # chunk runs must be 128-aligned
        L[k][ntile_of[k] - 1] += (-L[k].sum()) % 128
    seg_off = [np.zeros(ntile_of[k], np.int64) for k in range(NCH)]
    run_off = np.zeros(NCH + 1, np.int64)
    pos = 0
    for k in range(NCH):
        run_off[k] = pos
        for t in range(ntile_of[k]):
            seg_off[k][t] = pos
            pos += L[k][t]
    run_off[NCH] = pos
    E_pad = int(pos)
    assert E_pad % 128 == 0
    G = E_pad // 128

    # For 64-wide chunks: first non-empty chunk per 64-tile decides copy/add.
    # Tiles never touched by a 64-wide chunk are memset up front; the 128-wide
    # last chunk always accumulates with "add".
    n64 = NCH - 1 if NCH >= 2 else NCH
    first_k = np.full(NT, -1, np.int64)
    for t in range(NT):
        for k in range(n64):
            if L[k][t] > 0:
                first_k[t] = k
                break

    # ---- pairs (g, k, t, start, stop, wid), sorted by stream position
    pairs = []
    for k in range(NCH):
        for t in range(ntile_of[k]):
            if L[k][t] == 0:
                continue
            p0 = seg_off[k][t]; p1 = p0 + L[k][t]
            g0, g1 = p0 // 128, (p1 - 1) // 128
            for g in range(g0, g1 + 1):
                pairs.append((int(g), k, t, g == g0, g == g1, wid_of[k]))
    pairs.sort(key=lambda p: (p[0], seg_off[p[1]][p[2]]))
    NPAIRS = len(pairs)
    pair_g = np.array([p[0] for p in pairs], np.int64)
    pair_w = np.array([p[5] for p in pairs], np.int64)
    pair_off = np.concatenate([[0], np.cumsum(pair_w)]).astype(np.int64)
    SCOLS = int(pair_off[-1])

    # map every stream position to its pair id
    pair_of_pos = np.full(E_pad, -1, np.int64)
    for pi, (g, k, t, st, sp, w) in enumerate(pairs):
        a = max(g * 128, seg_off[k][t])
        b = min((g + 1) * 128, seg_off[k][t] + L[k][t])
        pair_of_pos[a:b] = pi

    # ---- per-core streams + S one-hot data
    percore_stream = []
    e_local = np.arange(E_pad) % 128
    for c in range(NC):
        idx_stream = np.zeros(E_pad, np.int16)
        scol = np.full(E_pad, -1, np.int16)
        ptr = np.zeros((NCH, NT), np.int64)
        # place this core's (already sorted) edges segment by segment
        kk, tt = ek[c], et[c]
        seg_id = kk * NT + tt
        # edges are sorted by (k,t) so contiguous runs per segment
        boundaries = np.nonzero(np.diff(seg_id))[0] + 1
        starts = np.concatenate([[0], boundaries])
        ends = np.concatenate([boundaries, [len(seg_id)]])
        for a, b in zip(starts, ends):
            if a == b:
                continue
            k, t = int(kk[a]), int(tt[a])
            o = seg_off[k][t]
            idx_stream[o:o + (b - a)] = esl[c][a:b]
            scol[o:o + (b - a)] = ecol[c][a:b]
        S = np.zeros((128, SCOLS), BF16_NP)
        valid = scol >= 0
        S[e_local[valid], pair_off[pair_of_pos[valid]] + scol[valid]] = 1
        percore_stream.append((idx_stream, S))

    # ---- gather calls
    calls = []  # (k, pos0, n, pair_lo, pair_hi)
    for k in range(NCH):
        a, b = int(run_off[k]), int(run_off[k + 1])
        for off in range(a, b, cfg.GCALL):
            n = min(cfg.GCALL, b - off)
            g0, g1 = off // 128, (off + n) // 128
            plo = int(np.searchsorted(pair_g, g0, "left"))
            phi = int(np.searchsorted(pair_g, g1, "left"))
            calls.append((k, off, n, plo, phi))

    # ---- decoder buckets
    li, lj = eli[0], eli[1]
    bi, bj, bperm = [], [], []
    bcounts = np.zeros((NC, NCH * NCH), np.int64)
    for c in range(NC):
        ii = remap(li[c * cfg.ELC:(c + 1) * cfg.ELC])
        jj = remap(lj[c * cfg.ELC:(c + 1) * cfg.ELC])
        bb = (ii // CHUNK) * NCH + (jj // CHUNK)
        order = np.argsort(bb, kind="stable")
        bi.append((ii % CHUNK)[order].astype(np.int16))
        bj.append((jj % CHUNK)[order].astype(np.int16))
        bperm.append(order + c * cfg.ELC)
        np.add.at(bcounts[c], bb[order], 1)
    LB = bcounts.max(axis=0)
    LB = ((LB + 127) // 128) * 128 * (LB > 0)
    ELPAD = int(LB.sum())
    bucket_off = np.concatenate([[0], np.cumsum(LB)]).astype(np.int64)

    deci, decj, perm = [], [], []
    for c in range(NC):
        di = np.zeros(ELPAD, np.int16)
        dj = np.zeros(ELPAD, np.int16)
        pm = np.full(ELPAD, -1, np.int64)
        csum = np.concatenate([[0], np.cumsum(bcounts[c])]).astype(np.int64)
        for b in range(NCH * NCH):
            nb = int(bcounts[c][b])
            if nb == 0:
                continue
            o = int(bucket_off[b])
            di[o:o + nb] = bi[c][csum[b]:csum[b] + nb]
            dj[o:o + nb] = bj[c][csum[b]:csum[b] + nb]
            pm[o:o + nb] = bperm[c][csum[b]:csum[b] + nb]
        deci.append(di); decj.append(dj); perm.append(pm)

    dbuckets = []  # (ci, cj, off, Lb)
    for b in range(NCH * NCH):
        if LB[b]:
            dbuckets.append((b // NCH, b % NCH, int(bucket_off[b]), int(LB[b])))

    # per-128-node-tile: the pair whose flush finalizes its accumulator
    NT2 = cfg.NT2
    stop_pair_of_seg = {}
    for pi, (g, k, t, st, sp, w) in enumerate(pairs):
        if sp:
            stop_pair_of_seg[(k, t)] = pi
    b3_after = {}   # pair index -> list of nt tiles finalized by that flush
    b3_empty = []   # tiles with no edges anywhere (memset path only)
    for nt in range(NT2):
        best = -1
        for k in range(NCH):
            if wid_of[k] == 128:
                if L[k][nt] > 0:
                    best = max(best, stop_pair_of_seg[(k, nt)])
            else:
                for t in (2 * nt, 2 * nt + 1):
                    if L[k][t] > 0:
                        best = max(best, stop_pair_of_seg[(k, t)])
        if best < 0:
            b3_empty.append(nt)
        else:
            b3_after.setdefault(best, []).append(nt)

    sched = dict(E_pad=E_pad, G=G, NPAIRS=NPAIRS, pairs=pairs, calls=calls,
                 first_k=first_k, L=L, seg_off=seg_off, pair_off=pair_off,
                 SCOLS=SCOLS, ELPAD=ELPAD, dbuckets=dbuckets, db4=db4,
                 b3_after=b3_after, b3_empty=b3_empty)

    # ---- per-core device input dicts
    xpad = np.zeros((cfg.NPAD, cfg.DIN), f32)
    xpad[:N] = x
    percore = []
    for c in range(NC):
        d = {}
        cs = c * SHARD
        d["xT"] = np.ascontiguousarray(xpad[cs:cs + SHARD].T)          # [DIN, SHARD]
        d["dinv128"] = np.ascontiguousarray(
            dinv_pad[cs:cs + SHARD].reshape(cfg.NT2, 128).T)           # [128, NT2]
        d["gidx"] = _wrap_idx(percore_stream[c][0])                    # [128, E_pad/16]
        d["S"] = percore_stream[c][1]                                  # [128, NPAIRS*64]
        for l in range(3):
            d[f"Ws{l}"] = Ws[l]
            d[f"u{l}"] = np.tile(ub[l][None, :], (128, 1)).astype(f32)  # [128, H]
        d["dW1i"] = dW1i.astype(BF16_NP)
        d["dW1j"] = dW1j.astype(BF16_NP)
        d["dW2"] = dW2.astype(BF16_NP)
        d["dW3"] = dW3.astype(BF16_NP)
        d["dW4"] = dW4.astype(BF16_NP)
        d["du1"] = du1.reshape(-1, 1).astype(f32)
        d["du2"] = du2.reshape(-1, 1).astype(f32)
        d["du3"] = du3.reshape(-1, 1).astype(f32)
        d["deci"] = _wrap_idx(deci[c])
        d["decj"] = _wrap_idx(decj[c])
        d["ident"] = np.eye(128, dtype=f32)
        percore.append(d)

    return sched, percore, perm


def build(cfg, sched):
    NC, H, DIN, SHARD, NT2, CHUNK, NCH = (cfg.NC, cfg.H, cfg.DIN, cfg.SHARD,
                                          cfg.NT2, cfg.CHUNK, cfg.NCH)
    E_pad, NPAIRS, ELPAD = sched["E_pad"], sched["NPAIRS"], sched["ELPAD"]
    TILE = cfg.TILE
    H2 = 2 * H

    nc = bacc.Bacc(None, target_bir_lowering=False, debug=False,
                   num_swdge_queues=4)

    # ---- dram parameters
    P = {}
    def di(name, shape, dtype):
        P[name] = nc.dram_tensor(name, list(shape), dtype, kind="ExternalInput")
    di("xT", (DIN, SHARD), F32)
    di("dinv128", (128, NT2), F32)
    di("gidx", (128, E_pad // 16), I16)
    di("S", (128, sched["SCOLS"]), BF16)
    di("Ws0", (DIN, H), F32); di("Ws1", (H, H), F32); di("Ws2", (H, H), F32)
    di("u0", (128, H), F32); di("u1", (128, H), F32); di("u2", (128, H), F32)
    di("dW1i", (H2, H2), BF16); di("dW1j", (H2, H2), BF16)
    di("dW2", (H2, H), BF16); di("dW3", (H, H // 2), BF16); di("dW4", (H // 2, 1), BF16)
    di("du1", (H2, 1), F32); di("du2", (H, 1), F32); di("du3", (H // 2, 1), F32)
    di("deci", (128, ELPAD // 16), I16)
    di("decj", (128, ELPAD // 16), I16)
    di("ident", (128, 128), F32)
    out = nc.dram_tensor("out", [1, ELPAD], F32, kind="ExternalOutput")

    bounce = nc.dram_tensor("bounce", [SHARD, 128], BF16)
    table = nc.dram_tensor("table", [cfg.NPAD, 128], BF16, addr_space="Shared")
    bounce_re = bounce.ap().rearrange("(t p) f -> p t f", p=128)  # [128, NT2, 128]

    def chunk_rows(k):
        lo = k * CHUNK
        hi = min((k + 1) * CHUNK, cfg.NPAD)
        return table[lo:hi, :]

    LBMAX = max((b[3] for b in sched["dbuckets"]), default=128)

    with tile.TileContext(nc) as tc:
        with tc.tile_pool(name="res", bufs=1) as res, \
             tc.tile_pool(name="sb", bufs=2) as sbp, \
             tc.tile_pool(name="sb3", bufs=4) as sbp3, \
             tc.tile_pool(name="sbz", bufs=3) as sbpz, \
             tc.tile_pool(name="pm", bufs=2, space="PSUM") as pmain, \
             tc.tile_pool(name="pae", bufs=3, space="PSUM") as pacc_e, \
             tc.tile_pool(name="pao", bufs=3, space="PSUM") as pacc_o:

            gq = [0]
            # ---- residents
            h_node = res.tile([128, NT2, H], F32)
            acc = res.tile([128, NT2, H], F32)
            hWloc = res.tile([128, NT2, H], F32)
            hW_bf = res.tile([128, NT2, 128], BF16)
            dinv_sb = res.tile([128, NT2], F32)
            gidx_sb = res.tile([128, E_pad // 16], I16)
            ident_sb = res.tile([128, 128], F32)
            Ws_sb = [res.tile([DIN if l == 0 else H, H], F32, name=f"Ws{l}_sb") for l in range(3)]
            u_sb = [res.tile([128, H], F32, name=f"u{l}_sb") for l in range(3)]
            dW1i_sb = res.tile([H2, H2], BF16)
            dW1j_sb = res.tile([H2, H2], BF16)
            dW2_sb = res.tile([H2, H], BF16)
            dW3_sb = res.tile([H, H // 2], BF16)
            dW4_sb = res.tile([H // 2, 1], BF16)
            du1_sb = res.tile([H2, 1], F32)
            du2_sb = res.tile([H, 1], F32)
            du3_sb = res.tile([H // 2, 1], F32)

            nc.sync.dma_start(out=dinv_sb[:], in_=P["dinv128"][:])
            nc.sync.dma_start(out=gidx_sb[:], in_=P["gidx"][:])
            nc.sync.dma_start(out=ident_sb[:], in_=P["ident"][:])
            for l in range(3):
                nc.sync.dma_start(out=Ws_sb[l][:], in_=P[f"Ws{l}"][:])
                nc.sync.dma_start(out=u_sb[l][:], in_=P[f"u{l}"][:])
            for t_, n_ in ((dW1i_sb, "dW1i"), (dW1j_sb, "dW1j"), (dW2_sb, "dW2"),
                           (dW3_sb, "dW3"), (dW4_sb, "dW4"), (du1_sb, "du1"),
                           (du2_sb, "du2"), (du3_sb, "du3")):
                nc.sync.dma_start(out=t_[:], in_=P[n_][:])
            nc.vector.memset(hW_bf[:], 0.0)

            ACT = mybir.ActivationFunctionType

            def finish_b0(nt, psB):
                nc.vector.tensor_scalar_mul(hWloc[:, nt, :], psB[:],
                                            dinv_sb[:, nt:nt + 1])
                nc.scalar.activation(hW_bf[:, nt, 0:H], hWloc[:, nt, :], ACT.Copy)

            first_k = sched["first_k"]

            def b0_tile(l_, nt):
                """hW'(l_) for 128-node tile nt (transpose path, l_ >= 1)."""
                psT = pmain.tile([H, 128], F32, tag="gen", name=f"psT{l_}_{nt}")
                nc.tensor.transpose(psT[:], h_node[:, nt, :], ident_sb[:])
                hTt = sbp.tile([H, 128], F32, tag="hTt", name=f"hTt{l_}_{nt}")
                nc.scalar.activation(hTt[:], psT[:], ACT.Copy)
                psB = pmain.tile([128, H], F32, tag="gen", name=f"psB{l_}_{nt}")
                nc.tensor.matmul(psB[:], hTt[:], Ws_sb[l_][:],
                                 start=True, stop=True)
                finish_b0(nt, psB)

            def b3_tile(l_, nt):
                """Epilogue for 128-node tile nt of layer l_."""
                tmp = sbp.tile([128, H], F32, tag="ep", name=f"ep{l_}_{nt}")
                nc.vector.tensor_add(tmp[:], acc[:, nt, :], hWloc[:, nt, :])
                nc.vector.tensor_scalar_mul(tmp[:], tmp[:], dinv_sb[:, nt:nt + 1])
                nc.vector.tensor_add(tmp[:], tmp[:], u_sb[l_][:])
                if l_ < 2:
                    nc.vector.tensor_scalar_max(h_node[:, nt, :], tmp[:], 0.0)
                else:
                    nc.scalar.activation(hW_bf[:, nt, 0:H], tmp[:], ACT.Copy)

            fin_cnt = [0]

            def finalize_tile(l_, nt):
                b3_tile(l_, nt)
                if l_ < 2:
                    b0_tile(l_ + 1, nt)
                if nt < NT2 // 2:
                    fin_cnt[0] += 1
                    if fin_cnt[0] == NT2 // 2:
                        ag_half(0)  # next table's first half is ready

            # ---- B0 layer 0: streamed xT slabs (emitted after ag_half is
            # defined below via a deferred list)
            _layer0_b0 = []

            def emit_layer0_b0():
                SLAB = 5  # slabs of SLAB 128-tiles
                for s0 in range(0, NT2, SLAB):
                    sw = min(SLAB, NT2 - s0)
                    xsl = sbp.tile([DIN, 5 * 128], F32, tag="big",
                                   name=f"xsl{s0}")
                    nc.sync.dma_start(out=xsl[:, 0:sw * 128],
                                      in_=P["xT"][:, s0 * 128:(s0 + sw) * 128])
                    for j in range(sw):
                        nt = s0 + j
                        psB = pmain.tile([128, H], F32, tag="gen",
                                         name=f"psB0_{nt}")
                        nc.tensor.matmul(psB[:], xsl[:, j * 128:(j + 1) * 128],
                                         Ws_sb[0][:], start=True, stop=True)
                        finish_b0(nt, psB)
                        if nt == NT2 // 2 - 1:
                            ag_half(0)
                ag_half(1)

            b3_after, b3_empty = sched["b3_after"], sched["b3_empty"]
            NTH = NT2 // 2          # tiles in the first table half
            S2R = NTH * 128         # bounce rows per half
            NP2 = cfg.NPAD // 2

            def ag_half(h):
                lo = h * NTH
                hi = NT2 if h else NTH
                nc.sync.dma_start(out=bounce_re[:, lo:hi, :],
                                  in_=hW_bf[:, lo:hi, :])
                nc.gpsimd.collective_compute(
                    "AllGather", mybir.AluOpType.bypass,
                    replica_groups=[list(range(NC))],
                    ins=[bounce[h * S2R:h * S2R + (hi - lo) * 128, :].opt()],
                    outs=[table[h * NP2:h * NP2 + NC * (hi - lo) * 128, :].opt()])

            emit_layer0_b0()

            for l in range(3):
                # (the table AGs for this layer were issued during the previous
                # layer's B2 / the layer-0 slab loop)
                # zero accumulator halves never touched by a 64-wide chunk
                for t in range(cfg.NT):
                    if first_k[t] < 0:
                        half, nt = t & 1, t >> 1
                        nc.vector.memset(acc[half * 64:half * 64 + 64, nt, :], 0.0)
                fin_cnt[0] = 0
                for nt in b3_empty:  # tiles with no edges at all
                    finalize_tile(l, nt)

                # ---- B2: message passing (B3 + next-layer B0 fire per tile as
                # soon as its accumulator is final, overlapping the B2 tail)
                cur = {}
                for ci_, (k, pos0, n, plo, phi) in enumerate(sched["calls"]):
                    msg = sbp3.tile([128, cfg.GCALL // 128, 128], BF16, tag="msg")
                    nc.gpsimd.dma_gather(
                        out_ap=msg[:, 0:n // 128, :], in_ap=chunk_rows(k),
                        idxs_ap=gidx_sb[:, pos0 // 16:(pos0 + n) // 16],
                        num_idxs=n, num_idxs_reg=n,
                        elem_size=128, single_packet=False,
                        queue_num=gq[0] % 4); gq[0] += 1
                    SWCOLS = 56 * TILE  # S-window column budget
                    po = sched["pair_off"]
                    win_lo = plo
                    Ssb = None
                    col_lo = 0
                    for li_ in range(phi - plo):
                        pi = plo + li_
                        g, k2, t, st, sp, wid = sched["pairs"][pi]
                        if Ssb is None or int(po[pi + 1] - col_lo) > SWCOLS:
                            col_lo = int(po[pi])
                            # window covers pairs [pi, wend)
                            wend = pi
                            while (wend < phi
                                   and int(po[wend + 1] - col_lo) <= SWCOLS):
                                wend += 1
                            ncols = int(po[wend] - col_lo)
                            Ssb = sbp.tile([128, SWCOLS], BF16, tag="Swin",
                                           name=f"Ssb{pi}")
                            nc.sync.dma_start(out=Ssb[:, 0:ncols],
                                              in_=P["S"][:, col_lo:col_lo + ncols])
                        gl = g - pos0 // 128
                        so = int(po[pi] - col_lo)
                        if wid == 64:
                            half, nt = t & 1, t >> 1
                            if ("h", t) not in cur:
                                pool_ = pacc_e if half == 0 else pacc_o
                                cur[("h", t)] = pool_.tile(
                                    [128, H], F32, name=f"pacc{t}",
                                    tag="ae" if half == 0 else "ao")
                            ps = cur[("h", t)]
                            pss = ps[half * 64:half * 64 + 64, :]
                            nc.tensor.matmul(pss, Ssb[:, so:so + 64],
                                             msg[:, gl, 0:H], start=st, stop=sp)
                            if sp:
                                asl = acc[half * 64:half * 64 + 64, nt, :]
                                if first_k[t] == k:
                                    nc.scalar.activation(asl, pss, ACT.Copy)
                                else:
                                    nc.vector.tensor_add(asl, asl, pss)
                                del cur[("h", t)]
                                for nt_ in b3_after.get(pi, []):
                                    finalize_tile(l, nt_)
                        else:
                            if ("f", t) not in cur:
                                cur[("f", t)] = pacc_e.tile(
                                    [128, H], F32, name=f"paccf{t}", tag="ae")
                            ps = cur[("f", t)]
                            nc.tensor.matmul(ps[:], Ssb[:, so:so + 128],
                                             msg[:, gl, 0:H], start=st, stop=sp)
                            if sp:
                                nc.vector.tensor_add(acc[:, t, :], acc[:, t, :],
                                                     ps[:])
                                del cur[("f", t)]
                                for nt_ in b3_after.get(pi, []):
                                    finalize_tile(l, nt_)
                assert not cur
                ag_half(1)  # next table's second half

            # ---- decoder
            for bi_, (ci, cj, off, Lb) in enumerate(sched["dbuckets"]):
                zti = sbp.tile([128, 1, LBMAX], BF16, tag="zti", name=f"zti{bi_}")
                ztj = sbp.tile([128, 1, LBMAX], BF16, tag="ztj", name=f"ztj{bi_}")
                DGC = 2048
                for zt, idx_dram, ck in ((zti, P["deci"], ci), (ztj, P["decj"], cj)):
                    for s0 in range(0, Lb, DGC):
                        n0 = min(DGC, Lb - s0)
                        isl = sbp3.tile([128, DGC // 16], I16, tag="gidxw",
                                        name=f"isl{off}_{s0}")
                        nc.sync.dma_start(
                            out=isl[:, 0:n0 // 16],
                            in_=idx_dram[:, (off + s0) // 16:(off + s0 + n0) // 16])
                        nc.gpsimd.dma_gather(
                            out_ap=zt[:, :, s0:s0 + n0], in_ap=chunk_rows(ck),
                            idxs_ap=isl[:, 0:n0 // 16], num_idxs=n0, num_idxs_reg=n0,
                            elem_size=128, transpose=True, single_packet=False,
                            queue_num=gq[0] % 4); gq[0] += 1
                for s in range(0, Lb, cfg.DSTEP):
                    w = min(cfg.DSTEP, Lb - s)
                    ps1 = pmain.tile([128, cfg.DSTEP], F32, tag="gen")
                    nc.tensor.matmul(ps1[:, 0:w], dW1i_sb[:], zti[:, 0, s:s + w],
                                     start=True, stop=False)
                    nc.tensor.matmul(ps1[:, 0:w], dW1j_sb[:], ztj[:, 0, s:s + w],
                                     start=False, stop=True)
                    a1 = sbp.tile([128, cfg.DSTEP], BF16, tag="a1")
                    nc.scalar.activation(a1[:, 0:w], ps1[:, 0:w], ACT.Relu,
                                         bias=du1_sb[:], scale=1.0)
                    ps2 = pmain.tile([H, cfg.DSTEP], F32, tag="gen")
                    nc.tensor.matmul(ps2[:, 0:w], dW2_sb[:], a1[:, 0:w],
                                     start=True, stop=True)
                    a2 = sbp.tile([H, cfg.DSTEP], BF16, tag="a2")
                    nc.scalar.activation(a2[:, 0:w], ps2[:, 0:w], ACT.Relu,
                                         bias=du2_sb[:], scale=1.0)
                    ps3 = pmain.tile([H // 2, cfg.DSTEP], F32, tag="gen")
                    nc.tensor.matmul(ps3[:, 0:w], dW3_sb[:], a2[:, 0:w],
                                     start=True, stop=True)
                    a3 = sbp.tile([H // 2, cfg.DSTEP], BF16, tag="a3")
                    nc.scalar.activation(a3[:, 0:w], ps3[:, 0:w], ACT.Relu,
                                         bias=du3_sb[:], scale=1.0)
                    ps4 = pmain.tile([1, cfg.DSTEP], F32, tag="gen")
                    nc.tensor.matmul(ps4[:, 0:w], dW4_sb[:], a3[:, 0:w],
                                     start=True, stop=True)
                    o_ = sbp.tile([1, cfg.DSTEP], F32, tag="od")
                    nc.scalar.activation(o_[:, 0:w], ps4[:, 0:w], ACT.Copy,
                                         bias=float(sched["db4"]))
                    nc.sync.dma_start(out=out[0:1, off + s:off + s + w],
                                      in_=o_[:, 0:w])

    nc.compile()
    return nc


def unshard(cfg, sched, perm, results):
    res = np.zeros(cfg.EL, np.float32)
    for c in range(cfg.NC):
        o = np.asarray(results[c]["out"], np.float32).reshape(-1)
        mask = perm[c] >= 0
        res[perm[c][mask]] = o[mask]
    return res


def kernel(**inputs):
    from concourse import bass_utils
    cfg = CFG(**FULL_CFG)
    sched, percore, perm = prepare(cfg, inputs)
    nc = build(cfg, sched)
    res = bass_utils.run_bass_kernel_spmd(
        nc, percore, core_ids=list(range(cfg.NC)))
    return unshard(cfg, sched, perm, res.results)


if __name__ == "__main__":
    print("kernel module; use test.py")


# revision 28
# speedup vs baseline: 1.1442x; 1.0507x over previous
#!/usr/bin/env python3
"""Trainium2 Bass kernel for AdvancedGNNLinkPredictor (3-layer GCN + MLP edge decoder).

Strategy (8 NeuronCores, SPMD):
  - Nodes sharded contiguously across cores (12544 rows each, 128-padded).
  - Per layer: each core computes hW' = dinv * (h @ W_folded) for its shard,
    AllGathers a bf16 node-feature table [100352, 128], then performs edge
    message passing for edges whose dst it owns: dma_gather of source rows +
    segment-sum via one-hot S-matrix matmuls on the TensorEngine (PSUM
    accumulation), then a fused BN/bias/ReLU epilogue.
  - Edge segment-sum avoids dma_scatter_add entirely (its CCE read-modify-write
    races on duplicate indices on HW).
  - Decoder: label edges bucketed by (src-chunk, dst-chunk) so int16 gather
    indices fit; transpose-mode dma_gather produces feature-major z vectors
    fed straight into a 4-layer bf16 matmul MLP.
  - BatchNorm (eval mode) folded into weights/biases on the host.
"""
import sys
import numpy as np

for _p in ("/opt/trn_rl_repo", "/root/.axon_site/_ro/trn_rl_repo"):
    if _p not in sys.path:
        sys.path.append(_p)

from concourse import bass, bacc, tile, mybir  # noqa: E402

BF16_NP = mybir.dt.np(mybir.dt.bfloat16)
F32 = mybir.dt.float32
BF16 = mybir.dt.bfloat16
I16 = mybir.dt.int16


class CFG:
    def __init__(self, N, E, EL, DIN, H, NC=8, CHUNK=32768, TILE=64,
                 GCALL=2048, DSTEP=512, BN_EPS=1e-5):
        self.N, self.E, self.EL, self.DIN, self.H = N, E, EL, DIN, H
        self.NC, self.CHUNK, self.TILE = NC, CHUNK, TILE
        self.GCALL, self.DSTEP, self.BN_EPS = GCALL, DSTEP, BN_EPS
        self.SHARD = -(-N // NC // 128) * 128
        self.NPAD = NC * self.SHARD
        self.NT = self.SHARD // TILE       # 64-node tiles per core
        self.NT2 = self.SHARD // 128       # 128-node tiles per core
        self.NCH = -(-self.NPAD // CHUNK)  # chunks in the gathered table
        self.ELC = EL // NC
        assert EL % NC == 0
        assert self.SHARD % 128 == 0 and TILE == 64


FULL_CFG = dict(N=100_000, E=1_250_000, EL=200_000, DIN=128, H=64)


def _wrap_idx(vals):
    """[n] int -> [128, n/16] int16 SBUF gather-index layout (i -> [i%16,i//16],
    replicated 8x across partition groups for the Q7 cpus)."""
    n = len(vals)
    assert n % 16 == 0
    a = np.asarray(vals, np.int16).reshape(n // 16, 16).T.copy()
    return np.tile(a, (8, 1))


def prepare(cfg, inp):
    """Host-side preprocessing. Returns (sched, percore) where sched is the
    core-independent static schedule and percore is a list of per-core input
    dicts for the device kernel."""
    f32 = np.float32
    N, E, NC = cfg.N, cfg.E, cfg.NC
    SHARD, CHUNK, TILE, NT, NCH = cfg.SHARD, cfg.CHUNK, cfg.TILE, cfg.NT, cfg.NCH

    x = np.asarray(inp["x"], f32)
    ei = np.asarray(inp["edge_index"], np.int64)
    eli = np.asarray(inp["edge_label_index"], np.int64)

    # ---- fold BN into GCN weights
    Ws, ub = [], []
    for l in range(3):
        W = np.asarray(inp[f"W{l}"], f32)
        b = np.asarray(inp[f"b{l}"], f32)
        g = np.asarray(inp[f"g{l}"], f32)
        be = np.asarray(inp[f"be{l}"], f32)
        m = np.asarray(inp[f"m{l}"], f32)
        v = np.asarray(inp[f"v{l}"], f32)
        s = g / np.sqrt(v + cfg.BN_EPS)
        Ws.append((W * s[None, :]).astype(f32))
        ub.append(((b - m) * s + be).astype(f32))

    # ---- decoder folds
    pW1 = np.asarray(inp["pW1"], f32); pb1 = np.asarray(inp["pb1"], f32)
    pW2 = np.asarray(inp["pW2"], f32); pb2 = np.asarray(inp["pb2"], f32)
    pW3 = np.asarray(inp["pW3"], f32); pb3 = np.asarray(inp["pb3"], f32)
    pW4 = np.asarray(inp["pW4"], f32); pb4 = np.asarray(inp["pb4"], f32)
    s1 = np.asarray(inp["pg1"], f32) / np.sqrt(np.asarray(inp["pv1"], f32) + cfg.BN_EPS)
    s2 = np.asarray(inp["pg2"], f32) / np.sqrt(np.asarray(inp["pv2"], f32) + cfg.BN_EPS)
    dW1 = pW1 * s1[None, :]
    du1 = (pb1 - np.asarray(inp["pm1"], f32)) * s1 + np.asarray(inp["pbe1"], f32)
    dW2 = pW2 * s2[None, :]
    du2 = (pb2 - np.asarray(inp["pm2"], f32)) * s2 + np.asarray(inp["pbe2"], f32)
    dW3, du3 = pW3, pb3
    dW4, db4 = pW4, float(pb4.reshape(-1)[0])
    H2 = 2 * cfg.H
    dW1i = np.zeros((H2, H2), f32); dW1i[:cfg.H] = dW1[:cfg.H]
    dW1j = np.zeros((H2, H2), f32); dW1j[:cfg.H] = dW1[cfg.H:]

    # ---- degrees
    src, dst = ei[0], ei[1]
    deg = 1.0 + np.bincount(dst, minlength=N).astype(f32)
    dinv = (1.0 / np.sqrt(deg)).astype(f32)
    dinv_pad = np.zeros(cfg.NPAD, f32)
    dinv_pad[:N] = dinv

    # ---- table-row remap: row = (m >= SHARD/2)*NPAD/2 + owner*SHARD/2 + m%(SHARD/2)
    # so the AllGather can be split into two half-shard collectives whose
    # outputs are contiguous table halves (half A = rows [0, NPAD/2)).
    S2 = SHARD // 2

    def remap(n):
        r, m = n // SHARD, n % SHARD
        return (m >= S2) * (cfg.NPAD // 2) + r * S2 + (m % S2)

    # ---- per-core edge lists sorted by (chunk, tile, src)
    # Chunks 0..NCH-2 aggregate into 64-node tiles; the last chunk (few source
    # rows -> tiny segments) uses 128-node tiles to halve pair count.
    wid_of = [128] * NCH
    ntile_of = [SHARD // w for w in wid_of]
    owner = dst // SHARD
    ek, et, esl, ecol = [], [], [], []
    counts = [np.zeros((NC, ntile_of[k]), np.int64) for k in range(NCH)]
    for c in range(NC):
        sel = owner == c
        s = remap(src[sel])
        dl = dst[sel] - c * SHARD
        k = s // CHUNK
        w = np.array(wid_of, np.int64)[k]
        t = dl // w
        col = dl % w
        order = np.lexsort((s, t, k))
        ek.append(k[order]); et.append(t[order])
        esl.append((s % CHUNK)[order].astype(np.int16))
        ecol.append(col[order].astype(np.int16))
        for kk in range(NCH):
            m = k[order] == kk
            np.add.at(counts[kk][c], t[order][m], 1)

    L = [counts[k].max(axis=0) for k in range(NCH)]  # ragged [NCH][ntile]
    for k in range(NCH):    # chunk runs must be 128-aligned
        L[k][ntile_of[k] - 1] += (-L[k].sum()) % 128
    seg_off = [np.zeros(ntile_of[k], np.int64) for k in range(NCH)]
    run_off = np.zeros(NCH + 1, np.int64)
    pos = 0
    for k in range(NCH):
        run_off[k] = pos
        for t in range(ntile_of[k]):
            seg_off[k][t] = pos
            pos += L[k][t]
    run_off[NCH] = pos
    E_pad = int(pos)
    assert E_pad % 128 == 0
    G = E_pad // 128

    # For 64-wide chunks: first non-empty chunk per 64-tile decides copy/add.
    # Tiles never touched by a 64-wide chunk are memset up front; the 128-wide
    # last chunk always accumulates with "add".
    first_k = np.full(NT, -1, np.int64)   # 64-wide path (unused when all-128)
    NT2_ = SHARD // 128
    first_k128 = np.full(NT2_, -1, np.int64)
    for nt in range(NT2_):
        for k in range(NCH):
            if wid_of[k] == 128 and L[k][nt] > 0:
                first_k128[nt] = k
                break

    # ---- pairs (g, k, t, start, stop, wid), sorted by stream position
    pairs = []
    for k in range(NCH):
        for t in range(ntile_of[k]):
            if L[k][t] == 0:
                continue
            p0 = seg_off[k][t]; p1 = p0 + L[k][t]
            g0, g1 = p0 // 128, (p1 - 1) // 128
            for g in range(g0, g1 + 1):
                pairs.append((int(g), k, t, g == g0, g == g1, wid_of[k]))
    pairs.sort(key=lambda p: (p[0], seg_off[p[1]][p[2]]))
    NPAIRS = len(pairs)
    pair_g = np.array([p[0] for p in pairs], np.int64)
    pair_w = np.array([p[5] for p in pairs], np.int64)
    pair_off = np.concatenate([[0], np.cumsum(pair_w)]).astype(np.int64)
    SCOLS = int(pair_off[-1])

    # map every stream position to its pair id
    pair_of_pos = np.full(E_pad, -1, np.int64)
    for pi, (g, k, t, st, sp, w) in enumerate(pairs):
        a = max(g * 128, seg_off[k][t])
        b = min((g + 1) * 128, seg_off[k][t] + L[k][t])
        pair_of_pos[a:b] = pi

    # ---- per-core streams + S one-hot data
    percore_stream = []
    e_local = np.arange(E_pad) % 128
    for c in range(NC):
        idx_stream = np.zeros(E_pad, np.int16)
        scol = np.full(E_pad, -1, np.int16)
        ptr = np.zeros((NCH, NT), np.int64)
        # place this core's (already sorted) edges segment by segment
        kk, tt = ek[c], et[c]
        seg_id = kk * NT + tt
        # edges are sorted by (k,t) so contiguous runs per segment
        boundaries = np.nonzero(np.diff(seg_id))[0] + 1
        starts = np.concatenate([[0], boundaries])
        ends = np.concatenate([boundaries, [len(seg_id)]])
        for a, b in zip(starts, ends):
            if a == b:
                continue
            k, t = int(kk[a]), int(tt[a])
            o = seg_off[k][t]
            idx_stream[o:o + (b - a)] = esl[c][a:b]
            scol[o:o + (b - a)] = ecol[c][a:b]
        S = np.zeros((128, SCOLS), BF16_NP)
        valid = scol >= 0
        S[e_local[valid], pair_off[pair_of_pos[valid]] + scol[valid]] = 1
        percore_stream.append((idx_stream, S))

    # ---- gather calls
    calls = []  # (k, pos0, n, pair_lo, pair_hi)
    for k in range(NCH):
        a, b = int(run_off[k]), int(run_off[k + 1])
        for off in range(a, b, cfg.GCALL):
            n = min(cfg.GCALL, b - off)
            g0, g1 = off // 128, (off + n) // 128
            plo = int(np.searchsorted(pair_g, g0, "left"))
            phi = int(np.searchsorted(pair_g, g1, "left"))
            calls.append((k, off, n, plo, phi))

    # ---- decoder buckets
    li, lj = eli[0], eli[1]
    bi, bj, bperm = [], [], []
    bcounts = np.zeros((NC, NCH * NCH), np.int64)
    for c in range(NC):
        ii = remap(li[c * cfg.ELC:(c + 1) * cfg.ELC])
        jj = remap(lj[c * cfg.ELC:(c + 1) * cfg.ELC])
        bb = (ii // CHUNK) * NCH + (jj // CHUNK)
        order = np.argsort(bb, kind="stable")
        bi.append((ii % CHUNK)[order].astype(np.int16))
        bj.append((jj % CHUNK)[order].astype(np.int16))
        bperm.append(order + c * cfg.ELC)
        np.add.at(bcounts[c], bb[order], 1)
    LB = bcounts.max(axis=0)
    LB = ((LB + 127) // 128) * 128 * (LB > 0)
    ELPAD = int(LB.sum())
    bucket_off = np.concatenate([[0], np.cumsum(LB)]).astype(np.int64)

    deci, decj, perm = [], [], []
    for c in range(NC):
        di = np.zeros(ELPAD, np.int16)
        dj = np.zeros(ELPAD, np.int16)
        pm = np.full(ELPAD, -1, np.int64)
        csum = np.concatenate([[0], np.cumsum(bcounts[c])]).astype(np.int64)
        for b in range(NCH * NCH):
            nb = int(bcounts[c][b])
            if nb == 0:
                continue
            o = int(bucket_off[b])
            di[o:o + nb] = bi[c][csum[b]:csum[b] + nb]
            dj[o:o + nb] = bj[c][csum[b]:csum[b] + nb]
            pm[o:o + nb] = bperm[c][csum[b]:csum[b] + nb]
        deci.append(di); decj.append(dj); perm.append(pm)

    dbuckets = []  # (ci, cj, off, Lb)
    for b in range(NCH * NCH):
        if LB[b]:
            dbuckets.append((b // NCH, b % NCH, int(bucket_off[b]), int(LB[b])))

    # per-128-node-tile: the pair whose flush finalizes its accumulator
    NT2 = cfg.NT2
    stop_pair_of_seg = {}
    for pi, (g, k, t, st, sp, w) in enumerate(pairs):
        if sp:
            stop_pair_of_seg[(k, t)] = pi
    b3_after = {}   # pair index -> list of nt tiles finalized by that flush
    b3_empty = []   # tiles with no edges anywhere (memset path only)
    for nt in range(NT2):
        best = -1
        for k in range(NCH):
            if wid_of[k] == 128:
                if L[k][nt] > 0:
                    best = max(best, stop_pair_of_seg[(k, nt)])
            else:
                for t in (2 * nt, 2 * nt + 1):
                    if L[k][t] > 0:
                        best = max(best, stop_pair_of_seg[(k, t)])
        if best < 0:
            b3_empty.append(nt)
        else:
            b3_after.setdefault(best, []).append(nt)

    sched = dict(E_pad=E_pad, G=G, NPAIRS=NPAIRS, pairs=pairs, calls=calls,
                 first_k=first_k, first_k128=first_k128,
                 L=L, seg_off=seg_off, pair_off=pair_off,
                 SCOLS=SCOLS, ELPAD=ELPAD, dbuckets=dbuckets, db4=db4,
                 b3_after=b3_after, b3_empty=b3_empty)

    # ---- per-core device input dicts
    xpad = np.zeros((cfg.NPAD, cfg.DIN), f32)
    xpad[:N] = x
    percore = []
    for c in range(NC):
        d = {}
        cs = c * SHARD
        d["xT"] = np.ascontiguousarray(xpad[cs:cs + SHARD].T)          # [DIN, SHARD]
        d["dinv128"] = np.ascontiguousarray(
            dinv_pad[cs:cs + SHARD].reshape(cfg.NT2, 128).T)           # [128, NT2]
        d["gidx"] = _wrap_idx(percore_stream[c][0])                    # [128, E_pad/16]
        d["S"] = percore_stream[c][1]                                  # [128, NPAIRS*64]
        for l in range(3):
            d[f"Ws{l}"] = Ws[l]
            d[f"u{l}"] = np.tile(ub[l][None, :], (128, 1)).astype(f32)  # [128, H]
        d["dW1i"] = dW1i.astype(BF16_NP)
        d["dW1j"] = dW1j.astype(BF16_NP)
        d["dW2"] = dW2.astype(BF16_NP)
        d["dW3"] = dW3.astype(BF16_NP)
        d["dW4"] = dW4.astype(BF16_NP)
        d["du1"] = du1.reshape(-1, 1).astype(f32)
        d["du2"] = du2.reshape(-1, 1).astype(f32)
        d["du3"] = du3.reshape(-1, 1).astype(f32)
        d["deci"] = _wrap_idx(deci[c])
        d["decj"] = _wrap_idx(decj[c])
        d["ident"] = np.eye(128, dtype=f32)
        percore.append(d)

    return sched, percore, perm


def build(cfg, sched):
    NC, H, DIN, SHARD, NT2, CHUNK, NCH = (cfg.NC, cfg.H, cfg.DIN, cfg.SHARD,
                                          cfg.NT2, cfg.CHUNK, cfg.NCH)
    E_pad, NPAIRS, ELPAD = sched["E_pad"], sched["NPAIRS"], sched["ELPAD"]
    TILE = cfg.TILE
    H2 = 2 * H

    nc = bacc.Bacc(None, target_bir_lowering=False, debug=False,
                   num_swdge_queues=4)

    # ---- dram parameters
    P = {}
    def di(name, shape, dtype):
        P[name] = nc.dram_tensor(name, list(shape), dtype, kind="ExternalInput")
    di("xT", (DIN, SHARD), F32)
    di("dinv128", (128, NT2), F32)
    di("gidx", (128, E_pad // 16), I16)
    di("S", (128, sched["SCOLS"]), BF16)
    di("Ws0", (DIN, H), F32); di("Ws1", (H, H), F32); di("Ws2", (H, H), F32)
    di("u0", (128, H), F32); di("u1", (128, H), F32); di("u2", (128, H), F32)
    di("dW1i", (H2, H2), BF16); di("dW1j", (H2, H2), BF16)
    di("dW2", (H2, H), BF16); di("dW3", (H, H // 2), BF16); di("dW4", (H // 2, 1), BF16)
    di("du1", (H2, 1), F32); di("du2", (H, 1), F32); di("du3", (H // 2, 1), F32)
    di("deci", (128, ELPAD // 16), I16)
    di("decj", (128, ELPAD // 16), I16)
    di("ident", (128, 128), F32)
    out = nc.dram_tensor("out", [1, ELPAD], F32, kind="ExternalOutput")

    bounce = nc.dram_tensor("bounce", [SHARD, 128], BF16)
    table = nc.dram_tensor("table", [cfg.NPAD, 128], BF16, addr_space="Shared")
    bounce_re = bounce.ap().rearrange("(t p) f -> p t f", p=128)  # [128, NT2, 128]

    def chunk_rows(k):
        lo = k * CHUNK
        hi = min((k + 1) * CHUNK, cfg.NPAD)
        return table[lo:hi, :]

    LBMAX = max((b[3] for b in sched["dbuckets"]), default=128)

    with tile.TileContext(nc) as tc:
        with tc.tile_pool(name="res", bufs=1) as res, \
             tc.tile_pool(name="sb", bufs=2) as sbp, \
             tc.tile_pool(name="sb3", bufs=4) as sbp3, \
             tc.tile_pool(name="sbz", bufs=3) as sbpz, \
             tc.tile_pool(name="pm", bufs=2, space="PSUM") as pmain, \
             tc.tile_pool(name="pae", bufs=3, space="PSUM") as pacc_e, \
             tc.tile_pool(name="pao", bufs=3, space="PSUM") as pacc_o:

            gq = [0]
            # ---- residents
            h_node = res.tile([128, NT2, H], F32)
            acc = res.tile([128, NT2, H], F32)
            hWloc = res.tile([128, NT2, H], F32)
            hW_bf = res.tile([128, NT2, 128], BF16)
            dinv_sb = res.tile([128, NT2], F32)
            gidx_sb = res.tile([128, E_pad // 16], I16)
            ident_sb = res.tile([128, 128], F32)
            Ws_sb = [res.tile([DIN if l == 0 else H, H], F32, name=f"Ws{l}_sb") for l in range(3)]
            u_sb = [res.tile([128, H], F32, name=f"u{l}_sb") for l in range(3)]
            dW1i_sb = res.tile([H2, H2], BF16)
            dW1j_sb = res.tile([H2, H2], BF16)
            dW2_sb = res.tile([H2, H], BF16)
            dW3_sb = res.tile([H, H // 2], BF16)
            dW4_sb = res.tile([H // 2, 1], BF16)
            du1_sb = res.tile([H2, 1], F32)
            du2_sb = res.tile([H, 1], F32)
            du3_sb = res.tile([H // 2, 1], F32)

            nc.sync.dma_start(out=dinv_sb[:], in_=P["dinv128"][:])
            nc.sync.dma_start(out=gidx_sb[:], in_=P["gidx"][:])
            nc.sync.dma_start(out=ident_sb[:], in_=P["ident"][:])
            for l in range(3):
                nc.sync.dma_start(out=Ws_sb[l][:], in_=P[f"Ws{l}"][:])
                nc.sync.dma_start(out=u_sb[l][:], in_=P[f"u{l}"][:])
            for t_, n_ in ((dW1i_sb, "dW1i"), (dW1j_sb, "dW1j"), (dW2_sb, "dW2"),
                           (dW3_sb, "dW3"), (dW4_sb, "dW4"), (du1_sb, "du1"),
                           (du2_sb, "du2"), (du3_sb, "du3")):
                nc.sync.dma_start(out=t_[:], in_=P[n_][:])
            nc.vector.memset(hW_bf[:], 0.0)

            ACT = mybir.ActivationFunctionType

            def finish_b0(nt, psB):
                nc.vector.tensor_scalar_mul(hWloc[:, nt, :], psB[:],
                                            dinv_sb[:, nt:nt + 1])
                nc.scalar.activation(hW_bf[:, nt, 0:H], hWloc[:, nt, :], ACT.Copy)

            first_k = sched["first_k"]

            def b0_tile(l_, nt):
                """hW'(l_) for 128-node tile nt (transpose path, l_ >= 1)."""
                psT = pmain.tile([H, 128], F32, tag="gen", name=f"psT{l_}_{nt}")
                nc.tensor.transpose(psT[:], h_node[:, nt, :], ident_sb[:])
                hTt = sbp.tile([H, 128], F32, tag="hTt", name=f"hTt{l_}_{nt}")
                nc.scalar.activation(hTt[:], psT[:], ACT.Copy)
                psB = pmain.tile([128, H], F32, tag="gen", name=f"psB{l_}_{nt}")
                nc.tensor.matmul(psB[:], hTt[:], Ws_sb[l_][:],
                                 start=True, stop=True)
                finish_b0(nt, psB)

            def b3_tile(l_, nt):
                """Epilogue for 128-node tile nt of layer l_."""
                tmp = sbp.tile([128, H], F32, tag="ep", name=f"ep{l_}_{nt}")
                nc.vector.tensor_add(tmp[:], acc[:, nt, :], hWloc[:, nt, :])
                nc.vector.tensor_scalar_mul(tmp[:], tmp[:], dinv_sb[:, nt:nt + 1])
                nc.vector.tensor_add(tmp[:], tmp[:], u_sb[l_][:])
                if l_ < 2:
                    nc.vector.tensor_scalar_max(h_node[:, nt, :], tmp[:], 0.0)
                else:
                    nc.scalar.activation(hW_bf[:, nt, 0:H], tmp[:], ACT.Copy)

            fin_cnt = [0]

            def finalize_tile(l_, nt):
                b3_tile(l_, nt)
                if l_ < 2:
                    b0_tile(l_ + 1, nt)
                if nt < NT2 // 2:
                    fin_cnt[0] += 1
                    if fin_cnt[0] == NT2 // 2:
                        ag_half(0)  # next table's first half is ready

            # ---- B0 layer 0: streamed xT slabs (emitted after ag_half is
            # defined below via a deferred list)
            _layer0_b0 = []

            def emit_layer0_b0():
                SLAB = 5  # slabs of SLAB 128-tiles
                for s0 in range(0, NT2, SLAB):
                    sw = min(SLAB, NT2 - s0)
                    xsl = sbp.tile([DIN, 5 * 128], F32, tag="big",
                                   name=f"xsl{s0}")
                    nc.sync.dma_start(out=xsl[:, 0:sw * 128],
                                      in_=P["xT"][:, s0 * 128:(s0 + sw) * 128])
                    for j in range(sw):
                        nt = s0 + j
                        psB = pmain.tile([128, H], F32, tag="gen",
                                         name=f"psB0_{nt}")
                        nc.tensor.matmul(psB[:], xsl[:, j * 128:(j + 1) * 128],
                                         Ws_sb[0][:], start=True, stop=True)
                        finish_b0(nt, psB)
                        if nt == NT2 // 2 - 1:
                            ag_half(0)
                ag_half(1)

            b3_after, b3_empty = sched["b3_after"], sched["b3_empty"]
            NTH = NT2 // 2          # tiles in the first table half
            S2R = NTH * 128         # bounce rows per half
            NP2 = cfg.NPAD // 2

            def ag_half(h):
                lo = h * NTH
                hi = NT2 if h else NTH
                nc.sync.dma_start(out=bounce_re[:, lo:hi, :],
                                  in_=hW_bf[:, lo:hi, :])
                nc.gpsimd.collective_compute(
                    "AllGather", mybir.AluOpType.bypass,
                    replica_groups=[list(range(NC))],
                    ins=[bounce[h * S2R:h * S2R + (hi - lo) * 128, :].opt()],
                    outs=[table[h * NP2:h * NP2 + NC * (hi - lo) * 128, :].opt()])

            emit_layer0_b0()

            for l in range(3):
                # (the table AGs for this layer were issued during the previous
                # layer's B2 / the layer-0 slab loop)
                # zero accumulator halves never touched by a 64-wide chunk
                if any(pp[5] == 64 for pp in sched["pairs"]):
                    for t in range(cfg.NT):
                        if first_k[t] < 0:
                            half, nt = t & 1, t >> 1
                            nc.vector.memset(
                                acc[half * 64:half * 64 + 64, nt, :], 0.0)
                fin_cnt[0] = 0
                for nt in b3_empty:  # tiles with no edges at all
                    finalize_tile(l, nt)

                # ---- B2: message passing (B3 + next-layer B0 fire per tile as
                # soon as its accumulator is final, overlapping the B2 tail)
                cur = {}
                for ci_, (k, pos0, n, plo, phi) in enumerate(sched["calls"]):
                    msg = sbp3.tile([128, cfg.GCALL // 128, 128], BF16, tag="msg")
                    nc.gpsimd.dma_gather(
                        out_ap=msg[:, 0:n // 128, :], in_ap=chunk_rows(k),
                        idxs_ap=gidx_sb[:, pos0 // 16:(pos0 + n) // 16],
                        num_idxs=n, num_idxs_reg=n,
                        elem_size=128, single_packet=False,
                        queue_num=gq[0] % 4); gq[0] += 1
                    SWCOLS = 56 * TILE  # S-window column budget
                    po = sched["pair_off"]
                    win_lo = plo
                    Ssb = None
                    col_lo = 0
                    for li_ in range(phi - plo):
                        pi = plo + li_
                        g, k2, t, st, sp, wid = sched["pairs"][pi]
                        if Ssb is None or int(po[pi + 1] - col_lo) > SWCOLS:
                            col_lo = int(po[pi])
                            # window covers pairs [pi, wend)
                            wend = pi
                            while (wend < phi
                                   and int(po[wend + 1] - col_lo) <= SWCOLS):
                                wend += 1
                            ncols = int(po[wend] - col_lo)
                            Ssb = sbp.tile([128, SWCOLS], BF16, tag="Swin",
                                           name=f"Ssb{pi}")
                            nc.sync.dma_start(out=Ssb[:, 0:ncols],
                                              in_=P["S"][:, col_lo:col_lo + ncols])
                        gl = g - pos0 // 128
                        so = int(po[pi] - col_lo)
                        if wid == 64:
                            half, nt = t & 1, t >> 1
                            if ("h", t) not in cur:
                                pool_ = pacc_e if half == 0 else pacc_o
                                cur[("h", t)] = pool_.tile(
                                    [128, H], F32, name=f"pacc{t}",
                                    tag="ae" if half == 0 else "ao")
                            ps = cur[("h", t)]
                            pss = ps[half * 64:half * 64 + 64, :]
                            nc.tensor.matmul(pss, Ssb[:, so:so + 64],
                                             msg[:, gl, 0:H], start=st, stop=sp)
                            if sp:
                                asl = acc[half * 64:half * 64 + 64, nt, :]
                                if first_k[t] == k:
                                    nc.scalar.activation(asl, pss, ACT.Copy)
                                else:
                                    nc.vector.tensor_add(asl, asl, pss)
                                del cur[("h", t)]
                                for nt_ in b3_after.get(pi, []):
                                    finalize_tile(l, nt_)
                        else:
                            if ("f", t) not in cur:
                                pool_ = pacc_e if (t & 1) == 0 else pacc_o
                                cur[("f", t)] = pool_.tile(
                                    [128, H], F32, name=f"paccf{t}",
                                    tag="ae" if (t & 1) == 0 else "ao")
                            ps = cur[("f", t)]
                            nc.tensor.matmul(ps[:], Ssb[:, so:so + 128],
                                             msg[:, gl, 0:H], start=st, stop=sp)
                            if sp:
                                if sched["first_k128"][t] == k:
                                    nc.scalar.activation(acc[:, t, :], ps[:],
                                                         ACT.Copy)
                                else:
                                    nc.vector.tensor_add(acc[:, t, :],
                                                         acc[:, t, :], ps[:])
                                del cur[("f", t)]
                                for nt_ in b3_after.get(pi, []):
                                    finalize_tile(l, nt_)
                assert not cur
                ag_half(1)  # next table's second half

            # ---- decoder
            for bi_, (ci, cj, off, Lb) in enumerate(sched["dbuckets"]):
                zti = sbp.tile([128, 1, LBMAX], BF16, tag="zti", name=f"zti{bi_}")
                ztj = sbp.tile([128, 1, LBMAX], BF16, tag="ztj", name=f"ztj{bi_}")
                DGC = 2048
                for zt, idx_dram, ck in ((zti, P["deci"], ci), (ztj, P["decj"], cj)):
                    for s0 in range(0, Lb, DGC):
                        n0 = min(DGC, Lb - s0)
                        isl = sbp3.tile([128, DGC // 16], I16, tag="gidxw",
                                        name=f"isl{off}_{s0}")
                        nc.sync.dma_start(
                            out=isl[:, 0:n0 // 16],
                            in_=idx_dram[:, (off + s0) // 16:(off + s0 + n0) // 16])
                        nc.gpsimd.dma_gather(
                            out_ap=zt[:, :, s0:s0 + n0], in_ap=chunk_rows(ck),
                            idxs_ap=isl[:, 0:n0 // 16], num_idxs=n0, num_idxs_reg=n0,
                            elem_size=128, transpose=True, single_packet=False,
                            queue_num=gq[0] % 4); gq[0] += 1
                for s in range(0, Lb, cfg.DSTEP):
                    w = min(cfg.DSTEP, Lb - s)
                    ps1 = pmain.tile([128, cfg.DSTEP], F32, tag="gen")
                    nc.tensor.matmul(ps1[:, 0:w], dW1i_sb[:], zti[:, 0, s:s + w],
                                     start=True, stop=False)
                    nc.tensor.matmul(ps1[:, 0:w], dW1j_sb[:], ztj[:, 0, s:s + w],
                                     start=False, stop=True)
                    a1 = sbp.tile([128, cfg.DSTEP], BF16, tag="a1")
                    nc.scalar.activation(a1[:, 0:w], ps1[:, 0:w], ACT.Relu,
                                         bias=du1_sb[:], scale=1.0)
                    ps2 = pmain.tile([H, cfg.DSTEP], F32, tag="gen")
                    nc.tensor.matmul(ps2[:, 0:w], dW2_sb[:], a1[:, 0:w],
                                     start=True, stop=True)
                    a2 = sbp.tile([H, cfg.DSTEP], BF16, tag="a2")
                    nc.scalar.activation(a2[:, 0:w], ps2[:, 0:w], ACT.Relu,
                                         bias=du2_sb[:], scale=1.0)
                    ps3 = pmain.tile([H // 2, cfg.DSTEP], F32, tag="gen")
                    nc.tensor.matmul(ps3[:, 0:w], dW3_sb[:], a2[:, 0:w],
                                     start=True, stop=True)
                    a3 = sbp.tile([H // 2, cfg.DSTEP], BF16, tag="a3")
                    nc.scalar.activation(a3[:, 0:w], ps3[:, 0:w], ACT.Relu,
                                         bias=du3_sb[:], scale=1.0)
                    ps4 = pmain.tile([1, cfg.DSTEP], F32, tag="gen")
                    nc.tensor.matmul(ps4[:, 0:w], dW4_sb[:], a3[:, 0:w],
                                     start=True, stop=True)
                    o_ = sbp.tile([1, cfg.DSTEP], F32, tag="od")
                    nc.scalar.activation(o_[:, 0:w], ps4[:, 0:w], ACT.Copy,
                                         bias=float(sched["db4"]))
                    nc.sync.dma_start(out=out[0:1, off + s:off + s + w],
                                      in_=o_[:, 0:w])

    nc.compile()
    return nc


def unshard(cfg, sched, perm, results):
    res = np.zeros(cfg.EL, np.float32)
    for c in range(cfg.NC):
        o = np.asarray(results[c]["out"], np.float32).reshape(-1)
        mask = perm[c] >= 0
        res[perm[c][mask]] = o[mask]
    return res


def kernel(**inputs):
    from concourse import bass_utils
    cfg = CFG(**FULL_CFG)
    sched, percore, perm = prepare(cfg, inputs)
    nc = build(cfg, sched)
    res = bass_utils.run_bass_kernel_spmd(
        nc, percore, core_ids=list(range(cfg.NC)))
    return unshard(cfg, sched, perm, res.results)


if __name__ == "__main__":
    print("kernel module; use test.py")
